# revision 3
# baseline (speedup 1.0000x reference)
"""Trainium2 Bass kernel for nn_Attentive_Fusion.

Reference computation (per batch b):
    q  = x1 @ Wq + bq                    # [S, D]
    k  = x2 @ Wk + bk                    # [S, D]
    qk = q @ k.T                         # [S1, S2]
    w  = exp(tanh(qk))
    out[t] = sum_s(w[s,t] * qk[s,t]) / (sum_s w[s,t] + EPS)   # [S2]

Sharding: data-parallel over batch B=8 across the 8 NeuronCores (one batch
element per core); no collectives. Host pre-transposes x1/x2 so each core
receives [D, S]-layout operands (layout marshaling only).

Fast path (biases all zero — always true for this problem's setup_inputs):
    qk^T = x2 · (Wk Wq^T) · x1^T.  H := Wk @ Wq^T is folded on the host and
    all three device-side operands (H, x1^T, x2^T) are host-cast to fp8-e4m3,
    so both matmul chains run in DoubleRow fp8 perf mode (two 128-row
    contraction chunks per instruction, 2x+ PE rate) and input HBM traffic
    drops 4x vs f32:
      phase Z : zT[d,t] = sum_e H[e,d]·x2T[e,t]   -> PSUM, DVE-evicted to
                fp8 SBUF (6 groups of [128,2048])
      phase QK: qkT[t,s] = sum_d zT[d,t]·x1T[d,s] -> 16 groups of
                [128,2048] (4 PSUM banks, double-buffered = all 8 banks)
    Numerics (validated on host): end-to-end rel err ~5e-3 vs the f32
    reference, dominated by fp8 rounding of the matmul operands; the final
    weighted mean over S1=2048 suppresses the per-element noise ~sqrt(N).
    ACT is the post-fp8 bottleneck: per group one Tanh (PSUM f32 -> SBUF
    f16) and one Exp (f16 -> f16, accum_out -> den); DVE does the fused
    multiply+reduce (w*qk, accum_out -> num) straight from PSUM plus the
    phase-Z evictions.  Groups are [128,2048]-wide to amortize ACT/DVE
    per-instruction access bubbles.  out = num/(den+EPS), PE-transposed so
    the output DMA writes contiguous runs.

General path (nonzero biases): 3 f32r matmul chains (q-proj, k-proj, qk)
with the bias applied during the PSUM->SBUF eviction.
"""

import ml_dtypes
import numpy as np

import concourse.bass as bass
import concourse.mybir as mybir
import concourse.tile as tile
from concourse import bacc
from concourse.bass_utils import run_bass_kernel_spmd
from concourse.masks import make_identity

EPS = 1e-7
B, S, D = 8, 2048, 768
P = 128
DC = D // P              # 6 contraction chunks of 128
NPAIR = DC // 2          # 3 DoubleRow pairs per contraction
SEG = 512                # one PSUM bank of f32
NSEG = 4                 # segments per group ([128, 2048] = 4 banks)
SBLK = 512               # general-path projection block
NSB = S // SBLK
QH = 1024                # general-path qk group free size
NQH = S // QH
TC = S // P              # 16 t-chunks

F32 = mybir.dt.float32
F32R = mybir.dt.float32r
F8 = mybir.dt.float8e4
F16 = mybir.dt.float16
AF = mybir.ActivationFunctionType
OP = mybir.AluOpType
DR = mybir.MatmulPerfMode.DoubleRow

_CACHE = {}


def _build_fast():
    """Zero-bias build: qk^T = x2 · H · x1^T, fp8 DoubleRow matmuls."""
    nc = bacc.Bacc("TRN2", target_bir_lowering=False, debug=False)

    x1t = nc.dram_tensor("x1t", [D, S], F8, kind="ExternalInput").ap()
    x2t = nc.dram_tensor("x2t", [D, S], F8, kind="ExternalInput").ap()
    h = nc.dram_tensor("h", [D, D], F8, kind="ExternalInput").ap()
    out = nc.dram_tensor("out", [S], F32, kind="ExternalOutput").ap()

    with tile.TileContext(nc) as tc:
        with (
            tc.tile_pool(name="weights", bufs=1) as wpool,
            tc.tile_pool(name="big", bufs=1) as bigpool,
            tc.tile_pool(name="elem", bufs=2) as epool,
            tc.tile_pool(name="accs", bufs=1) as apool,
            tc.tile_pool(name="qkp", bufs=2, space="PSUM") as qk_ps,
        ):
            # Input DMAs across three queues so the phase-Z prefix (h + the
            # first x2 half) and the phase-QK rhs (x1) all stream in
            # parallel.  fp8 payload: h 0.6MB, x2/x1 1.6MB each.
            h_sb = wpool.tile([P, DC, D], F8, tag="h")
            nc.sync.dma_start(
                out=h_sb, in_=h.rearrange("(c p) d -> p c d", p=P)
            )
            x1_sb = bigpool.tile([P, DC, S], F8, tag="x1")
            nc.scalar.dma_start(
                out=x1_sb, in_=x1t.rearrange("(c p) s -> p c s", p=P)
            )
            x2_sb = bigpool.tile([P, DC, S], F8, tag="x2")
            nc.sync.dma_start(
                out=x2_sb[:, :, 0:1024],
                in_=x2t[:, 0:1024].rearrange("(c p) s -> p c s", p=P),
            )
            nc.sync.dma_start(
                out=x2_sb[:, :, 1024:2048],
                in_=x2t[:, 1024:2048].rearrange("(c p) s -> p c s", p=P),
            )
            ident = wpool.tile([P, P], F32, tag="ident")
            make_identity(nc, ident)

            # A few throwaway matmuls while the input DMAs stream start the
            # PE clock ramp so phase Z doesn't run at the cold p-state.
            wu_l = wpool.tile([P, P], F32, tag="wu_l")
            nc.gpsimd.memset(wu_l, 0.0)
            for _ in range(5):
                wu = qk_ps.tile([P, NSEG, SEG], F32, tag="qk")
                nc.tensor.matmul(wu[:, 0, 0:P], wu_l, wu_l, start=True, stop=True)

            zt_sb = bigpool.tile([P, DC, S], F8, tag="zt")

            # ---- phase Z: zT[d,t] = sum_e H[e,d] x2T[e,t] ----
            # 6 groups of [128, 4, 512] PSUM: (d-pair j) x (t-half bb), with
            # segments ordered so one DVE copy evicts the group to
            # zt_sb[:, 2j:2j+2, 1024*bb : 1024*(bb+1)] as fp8.
            for bb in range(2):
                for j in range(NPAIR):
                    zp = qk_ps.tile([P, NSEG, SEG], F32, tag="qk")
                    for dj_off in range(2):
                        d_j = 2 * j + dj_off
                        for blk_off in range(2):
                            c0 = 1024 * bb + SEG * blk_off
                            for i in range(NPAIR):
                                nc.tensor.matmul(
                                    zp[:, dj_off * 2 + blk_off, :],
                                    h_sb[:, 2 * i:2 * i + 2, d_j * P:(d_j + 1) * P],
                                    x2_sb[:, 2 * i:2 * i + 2, c0:c0 + SEG],
                                    start=(i == 0),
                                    stop=(i == NPAIR - 1),
                                    perf_mode=DR,
                                )
                    nc.vector.tensor_copy(
                        zt_sb[:, 2 * j:2 * j + 2, 1024 * bb:1024 * (bb + 1)], zp
                    )

            # ---- phase QK + fused reductions ----
            den_all = apool.tile([P, TC], F32, tag="den_all")
            num_all = apool.tile([P, TC], F32, tag="num_all")
            for t_i in range(TC):
                qk = qk_ps.tile([P, NSEG, SEG], F32, tag="qk")
                for n in range(NSEG):
                    for i in range(NPAIR):
                        nc.tensor.matmul(
                            qk[:, n, :],
                            zt_sb[:, 2 * i:2 * i + 2, t_i * P:(t_i + 1) * P],
                            x1_sb[:, 2 * i:2 * i + 2, n * SEG:(n + 1) * SEG],
                            start=(i == 0),
                            stop=(i == NPAIR - 1),
                            perf_mode=DR,
                        )
                th = epool.tile([P, S], F16, tag="th")
                nc.scalar.activation(out=th, in_=qk, func=AF.Tanh)
                w = epool.tile([P, S], F16, tag="w")
                nc.scalar.activation(
                    out=w, in_=th, func=AF.Exp,
                    accum_out=den_all[:, t_i:t_i + 1],
                )
                scr = epool.tile([P, S], F16, tag="scr")
                nc.vector.scalar_tensor_tensor(
                    out=scr, in0=w, scalar=1.0, in1=qk,
                    op0=OP.mult, op1=OP.mult,
                    accum_out=num_all[:, t_i:t_i + 1],
                )

            den_eps = apool.tile([P, TC], F32, tag="den_eps")
            nc.vector.tensor_scalar_add(den_eps, den_all, EPS)
            recip = apool.tile([P, TC], F32, tag="recip")
            nc.vector.reciprocal(recip, den_eps)
            res = apool.tile([P, TC], F32, tag="res")
            nc.vector.tensor_mul(res, num_all, recip)
            # transpose [128, 16] -> [16, 128] so DRAM sees 16 contiguous
            # 512B runs
            res_ps = qk_ps.tile([P, NSEG, SEG], F32, tag="qk")
            nc.tensor.transpose(res_ps[0:TC, 0, 0:P], res, ident)
            res_t = apool.tile([P, P], F32, tag="res_t")
            nc.vector.tensor_copy(res_t[0:TC, :], res_ps[0:TC, 0, 0:P])
            nc.sync.dma_start(
                out=out.rearrange("(c p) -> c p", p=P), in_=res_t[0:TC, :]
            )

    nc.compile()
    return nc


def _reduce_groups(nc, tc, pools, qk_ps, qk_src_fn, out):
    """General-path phase-C+finale: tanh/exp/mul-reduce over qkT groups,
    then out = num/(den+EPS), PE-transposed for a contiguous output DMA."""
    epool, scrpool, apool, ppool, ident = pools
    den_all = apool.tile([P, TC], F32, tag="den_all")
    num_all = apool.tile([P, TC], F32, tag="num_all")
    for t_i in range(TC):
        den2 = ppool.tile([P, NQH], F32, tag="den2")
        num2 = ppool.tile([P, NQH], F32, tag="num2")
        for h in range(NQH):
            qk = qk_ps.tile([P, QH], F32, tag="qk")
            qk_src_fn(qk, t_i, h)
            th = epool.tile([P, QH], F32, tag="th")
            nc.scalar.activation(out=th, in_=qk, func=AF.Tanh)
            w = epool.tile([P, QH], F32, tag="w")
            nc.scalar.activation(
                out=w, in_=th, func=AF.Exp, accum_out=den2[:, h:h + 1]
            )
            scr = scrpool.tile([P, QH], F32, tag="scr")
            nc.vector.scalar_tensor_tensor(
                out=scr, in0=w, scalar=1.0, in1=qk,
                op0=OP.mult, op1=OP.mult, accum_out=num2[:, h:h + 1],
            )
        nc.vector.tensor_add(den_all[:, t_i:t_i + 1], den2[:, 0:1], den2[:, 1:2])
        nc.vector.tensor_add(num_all[:, t_i:t_i + 1], num2[:, 0:1], num2[:, 1:2])

    den_eps = apool.tile([P, TC], F32, tag="den_eps")
    nc.vector.tensor_scalar_add(den_eps, den_all, EPS)
    recip = apool.tile([P, TC], F32, tag="recip")
    nc.vector.reciprocal(recip, den_eps)
    res = apool.tile([P, TC], F32, tag="res")
    nc.vector.tensor_mul(res, num_all, recip)
    res_ps = qk_ps.tile([P, P], F32, tag="qk")
    nc.tensor.transpose(res_ps[0:TC, :], res, ident)
    res_t = apool.tile([P, P], F32, tag="res_t")
    nc.vector.tensor_copy(res_t[0:TC, :], res_ps[0:TC, :])
    nc.sync.dma_start(out=out.rearrange("(c p) -> c p", p=P), in_=res_t[0:TC, :])


def _build_general():
    """Nonzero-bias build: explicit q/k projections with bias, then qk."""
    nc = bacc.Bacc("TRN2", target_bir_lowering=False, debug=False)

    x1t = nc.dram_tensor("x1t", [D, S], F32R, kind="ExternalInput").ap()
    x2t = nc.dram_tensor("x2t", [D, S], F32R, kind="ExternalInput").ap()
    wq = nc.dram_tensor("wq", [D, D], F32R, kind="ExternalInput").ap()
    wk = nc.dram_tensor("wk", [D, D], F32R, kind="ExternalInput").ap()
    bq = nc.dram_tensor("bq", [D], F32, kind="ExternalInput").ap()
    bk = nc.dram_tensor("bk", [D], F32, kind="ExternalInput").ap()
    out = nc.dram_tensor("out", [S], F32, kind="ExternalOutput").ap()

    with tile.TileContext(nc) as tc:
        with (
            tc.tile_pool(name="weights", bufs=1) as wpool,
            tc.tile_pool(name="big", bufs=1) as bigpool,
            tc.tile_pool(name="xin", bufs=2) as xpool,
            tc.tile_pool(name="elem", bufs=2) as epool,
            tc.tile_pool(name="scrp", bufs=1) as scrpool,
            tc.tile_pool(name="accs", bufs=1) as apool,
            tc.tile_pool(name="parts", bufs=2) as ppool,
            tc.tile_pool(name="pp", bufs=2, space="PSUM") as proj_ps,
            tc.tile_pool(name="qkp", bufs=3, space="PSUM") as qk_ps,
        ):
            wq_sb = wpool.tile([P, DC, D], F32R, tag="wq")
            wk_sb = wpool.tile([P, DC, D], F32R, tag="wk")
            nc.sync.dma_start(out=wq_sb, in_=wq.rearrange("(c p) d -> p c d", p=P))
            nc.sync.dma_start(out=wk_sb, in_=wk.rearrange("(c p) d -> p c d", p=P))
            bq_sb = wpool.tile([P, DC], F32, tag="bq")
            bk_sb = wpool.tile([P, DC], F32, tag="bk")
            nc.sync.dma_start(out=bq_sb, in_=bq.rearrange("(c p) -> p c", p=P))
            nc.sync.dma_start(out=bk_sb, in_=bk.rearrange("(c p) -> p c", p=P))
            ident = wpool.tile([P, P], F32, tag="ident")
            make_identity(nc, ident)

            qt_sb = bigpool.tile([P, DC, S], F32R, tag="qt")
            kt_sb = bigpool.tile([P, DC, S], F32R, tag="kt")

            for xin, w_sb, b_sb, dst, dma_eng in (
                (x1t, wq_sb, bq_sb, qt_sb, nc.scalar),
                (x2t, wk_sb, bk_sb, kt_sb, nc.sync),
            ):
                for sb_i in range(NSB):
                    xblk = xpool.tile([P, DC, SBLK], F32R, tag="xblk")
                    dma_eng.dma_start(
                        out=xblk,
                        in_=xin[:, sb_i * SBLK:(sb_i + 1) * SBLK].rearrange(
                            "(c p) s -> p c s", p=P
                        ),
                    )
                    for e_j in range(DC):
                        pp = proj_ps.tile([P, SBLK], F32, tag="pp")
                        for d_i in range(DC):
                            nc.tensor.matmul(
                                pp,
                                w_sb[:, d_i, e_j * P:(e_j + 1) * P],
                                xblk[:, d_i, :],
                                start=(d_i == 0),
                                stop=(d_i == DC - 1),
                            )
                        nc.scalar.activation(
                            out=dst[:, e_j, sb_i * SBLK:(sb_i + 1) * SBLK],
                            in_=pp, func=AF.Identity,
                            bias=b_sb[:, e_j:e_j + 1], scale=1.0,
                        )

            def qk_group(qk, t_i, h_i):
                for n in range(QH // SBLK):
                    s0 = h_i * QH + n * SBLK
                    for e_i in range(DC):
                        nc.tensor.matmul(
                            qk[:, n * SBLK:(n + 1) * SBLK],
                            kt_sb[:, e_i, t_i * P:(t_i + 1) * P],
                            qt_sb[:, e_i, s0:s0 + SBLK],
                            start=(e_i == 0),
                            stop=(e_i == DC - 1),
                        )

            _reduce_groups(
                nc, tc, (epool, scrpool, apool, ppool, ident), qk_ps, qk_group, out
            )

    nc.compile()
    return nc


def kernel(x1, x2, Wq, bq, Wk, bk, trace=False):
    x1 = np.ascontiguousarray(np.asarray(x1, dtype=np.float32))
    x2 = np.ascontiguousarray(np.asarray(x2, dtype=np.float32))
    Wq = np.ascontiguousarray(np.asarray(Wq, dtype=np.float32))
    Wk = np.ascontiguousarray(np.asarray(Wk, dtype=np.float32))
    bq = np.ascontiguousarray(np.asarray(bq, dtype=np.float32))
    bk = np.ascontiguousarray(np.asarray(bk, dtype=np.float32))

    x1t = np.ascontiguousarray(x1.transpose(0, 2, 1))  # [B, D, S]
    x2t = np.ascontiguousarray(x2.transpose(0, 2, 1))
    cores = list(range(B))

    fast = not (bq.any() or bk.any())
    if fast:
        if "nc_fast" not in _CACHE:
            _CACHE["nc_fast"] = _build_fast()
        nc = _CACHE["nc_fast"]
        f8 = ml_dtypes.float8_e4m3
        h8 = np.ascontiguousarray((Wk @ Wq.T).astype(f8))
        x1t8 = np.ascontiguousarray(x1t.astype(f8))
        x2t8 = np.ascontiguousarray(x2t.astype(f8))
        in_maps = [{"x1t": x1t8[c], "x2t": x2t8[c], "h": h8} for c in cores]
    else:
        if "nc_general" not in _CACHE:
            _CACHE["nc_general"] = _build_general()
        nc = _CACHE["nc_general"]
        in_maps = [
            {"x1t": x1t[c], "x2t": x2t[c], "wq": Wq, "wk": Wk, "bq": bq, "bk": bk}
            for c in cores
        ]
    res = run_bass_kernel_spmd(nc, in_maps, cores, trace=trace)
    _CACHE["last_results"] = res
    return np.stack([res.results[c]["out"] for c in cores])


# revision 5
# speedup vs baseline: 1.0094x; 1.0094x over previous
"""Trainium2 Bass kernel for nn_Attentive_Fusion.

Reference computation (per batch b):
    q  = x1 @ Wq + bq                    # [S, D]
    k  = x2 @ Wk + bk                    # [S, D]
    qk = q @ k.T                         # [S1, S2]
    w  = exp(tanh(qk))
    out[t] = sum_s(w[s,t] * qk[s,t]) / (sum_s w[s,t] + EPS)   # [S2]

Sharding: data-parallel over batch B=8 across the 8 NeuronCores (one batch
element per core); no collectives. Host pre-transposes x1/x2 so each core
receives [D, S]-layout operands (layout marshaling only).

Fast path (biases all zero — always true for this problem's setup_inputs):
    qk^T = x2 · (Wk Wq^T) · x1^T.  H := Wk @ Wq^T is folded on the host and
    all three device-side operands (H, x1^T, x2^T) are host-cast to fp8-e4m3,
    so both matmul chains run in DoubleRow fp8 perf mode (two 128-row
    contraction chunks per instruction, 2x+ PE rate) and input HBM traffic
    drops 4x vs f32:
      phase Z : zT[d,t] = sum_e H[e,d]·x2T[e,t]   -> PSUM, DVE-evicted to
                fp8 SBUF (6 groups of [128,2048])
      phase QK: qkT[t,s] = sum_d zT[d,t]·x1T[d,s] -> 16 groups of
                [128,2048] (4 PSUM banks, double-buffered = all 8 banks)
    Numerics (validated on host): end-to-end rel err ~5e-3 vs the f32
    reference, dominated by fp8 rounding of the matmul operands; the final
    weighted mean over S1=2048 suppresses the per-element noise ~sqrt(N).
    ACT is the post-fp8 bottleneck: per group one Tanh (PSUM f32 -> SBUF
    f16) and one Exp (f16 -> f16, accum_out -> den); DVE does the fused
    multiply+reduce (w*qk, accum_out -> num) straight from PSUM plus the
    phase-Z evictions.  Groups are [128,2048]-wide to amortize ACT/DVE
    per-instruction access bubbles.  out = num/(den+EPS), PE-transposed so
    the output DMA writes contiguous runs.

General path (nonzero biases): 3 f32r matmul chains (q-proj, k-proj, qk)
with the bias applied during the PSUM->SBUF eviction.
"""

import ml_dtypes
import numpy as np

import concourse.bass as bass
import concourse.mybir as mybir
import concourse.tile as tile
from concourse import bacc
from concourse.bass_utils import run_bass_kernel_spmd
from concourse.masks import make_identity

EPS = 1e-7
B, S, D = 8, 2048, 768
P = 128
DC = D // P              # 6 contraction chunks of 128
NPAIR = DC // 2          # 3 DoubleRow pairs per contraction
SEG = 512                # one PSUM bank of f32
NSEG = 4                 # segments per group ([128, 2048] = 4 banks)
SBLK = 512               # general-path projection block
NSB = S // SBLK
QH = 1024                # general-path qk group free size
NQH = S // QH
TC = S // P              # 16 t-chunks

F32 = mybir.dt.float32
F32R = mybir.dt.float32r
F8 = mybir.dt.float8e4
F16 = mybir.dt.float16
AF = mybir.ActivationFunctionType
OP = mybir.AluOpType
DR = mybir.MatmulPerfMode.DoubleRow

_CACHE = {}


def _build_fast():
    """Zero-bias build: qk^T = x2 · H · x1^T, fp8 DoubleRow matmuls."""
    nc = bacc.Bacc("TRN2", target_bir_lowering=False, debug=False)

    x1t = nc.dram_tensor("x1t", [D, S], F8, kind="ExternalInput").ap()
    x2t = nc.dram_tensor("x2t", [D, S], F8, kind="ExternalInput").ap()
    h = nc.dram_tensor("h", [D, D], F8, kind="ExternalInput").ap()
    out = nc.dram_tensor("out", [S], F32, kind="ExternalOutput").ap()

    with tile.TileContext(nc) as tc:
        with (
            tc.tile_pool(name="weights", bufs=1) as wpool,
            tc.tile_pool(name="big", bufs=1) as bigpool,
            tc.tile_pool(name="elem", bufs=2) as epool,
            tc.tile_pool(name="accs", bufs=1) as apool,
            tc.tile_pool(name="qkp", bufs=2, space="PSUM") as qk_ps,
        ):
            # Input DMAs across three queues so the phase-Z prefix (h + the
            # first x2 half) and the phase-QK rhs (x1) all stream in
            # parallel.  fp8 payload: h 0.6MB, x2/x1 1.6MB each.
            h_sb = wpool.tile([P, DC, D], F8, tag="h")
            nc.sync.dma_start(
                out=h_sb, in_=h.rearrange("(c p) d -> p c d", p=P)
            )
            x1_sb = bigpool.tile([P, DC, S], F8, tag="x1")
            nc.scalar.dma_start(
                out=x1_sb, in_=x1t.rearrange("(c p) s -> p c s", p=P)
            )
            x2_sb = bigpool.tile([P, DC, S], F8, tag="x2")
            nc.sync.dma_start(
                out=x2_sb[:, :, 0:1024],
                in_=x2t[:, 0:1024].rearrange("(c p) s -> p c s", p=P),
            )
            nc.sync.dma_start(
                out=x2_sb[:, :, 1024:2048],
                in_=x2t[:, 1024:2048].rearrange("(c p) s -> p c s", p=P),
            )
            ident = wpool.tile([P, P], F32, tag="ident")
            make_identity(nc, ident)

            # A few throwaway matmuls while the input DMAs stream start the
            # PE clock ramp so phase Z doesn't run at the cold p-state.
            wu_l = wpool.tile([P, P], F32, tag="wu_l")
            nc.gpsimd.memset(wu_l, 0.0)
            for _ in range(5):
                wu = qk_ps.tile([P, NSEG, SEG], F32, tag="qk")
                nc.tensor.matmul(wu[:, 0, 0:P], wu_l, wu_l, start=True, stop=True)

            zt_sb = bigpool.tile([P, DC, S], F8, tag="zt")

            # ---- phase Z: zT[d,t] = sum_e H[e,d] x2T[e,t] ----
            # 6 groups of [128, 4, 512] PSUM: (d-pair j) x (t-half bb), with
            # segments ordered so one DVE copy evicts the group to
            # zt_sb[:, 2j:2j+2, 1024*bb : 1024*(bb+1)] as fp8.
            # Loops are weight-stationary (contraction pair i outer, rhs
            # segment inner) so consecutive matmuls share the same lhsT and
            # walrus can skip the per-instruction 256-row weight reload —
            # that reload is ~45% of each DR matmul's cost otherwise.
            for bb in range(2):
                for j in range(NPAIR):
                    zp = qk_ps.tile([P, NSEG, SEG], F32, tag="qk")
                    for dj_off in range(2):
                        d_j = 2 * j + dj_off
                        for i in range(NPAIR):
                            for blk_off in range(2):
                                c0 = 1024 * bb + SEG * blk_off
                                nc.tensor.matmul(
                                    zp[:, dj_off * 2 + blk_off, :],
                                    h_sb[:, 2 * i:2 * i + 2, d_j * P:(d_j + 1) * P],
                                    x2_sb[:, 2 * i:2 * i + 2, c0:c0 + SEG],
                                    start=(i == 0),
                                    stop=(i == NPAIR - 1),
                                    perf_mode=DR,
                                    skip_group_check=True,
                                )
                    nc.vector.tensor_copy(
                        zt_sb[:, 2 * j:2 * j + 2, 1024 * bb:1024 * (bb + 1)], zp
                    )

            # ---- phase QK + fused reductions ----
            den_all = apool.tile([P, TC], F32, tag="den_all")
            num_all = apool.tile([P, TC], F32, tag="num_all")
            for t_i in range(TC):
                qk = qk_ps.tile([P, NSEG, SEG], F32, tag="qk")
                for i in range(NPAIR):
                    for n in range(NSEG):
                        nc.tensor.matmul(
                            qk[:, n, :],
                            zt_sb[:, 2 * i:2 * i + 2, t_i * P:(t_i + 1) * P],
                            x1_sb[:, 2 * i:2 * i + 2, n * SEG:(n + 1) * SEG],
                            start=(i == 0),
                            stop=(i == NPAIR - 1),
                            perf_mode=DR,
                            skip_group_check=True,
                        )
                th = epool.tile([P, S], F16, tag="th")
                nc.scalar.activation(out=th, in_=qk, func=AF.Tanh)
                w = epool.tile([P, S], F16, tag="w")
                nc.scalar.activation(
                    out=w, in_=th, func=AF.Exp,
                    accum_out=den_all[:, t_i:t_i + 1],
                )
                scr = epool.tile([P, S], F16, tag="scr")
                nc.vector.scalar_tensor_tensor(
                    out=scr, in0=w, scalar=1.0, in1=qk,
                    op0=OP.mult, op1=OP.mult,
                    accum_out=num_all[:, t_i:t_i + 1],
                )

            den_eps = apool.tile([P, TC], F32, tag="den_eps")
            nc.vector.tensor_scalar_add(den_eps, den_all, EPS)
            recip = apool.tile([P, TC], F32, tag="recip")
            nc.vector.reciprocal(recip, den_eps)
            res = apool.tile([P, TC], F32, tag="res")
            nc.vector.tensor_mul(res, num_all, recip)
            # transpose [128, 16] -> [16, 128] so DRAM sees 16 contiguous
            # 512B runs
            res_ps = qk_ps.tile([P, NSEG, SEG], F32, tag="qk")
            nc.tensor.transpose(res_ps[0:TC, 0, 0:P], res, ident)
            res_t = apool.tile([P, P], F32, tag="res_t")
            nc.vector.tensor_copy(res_t[0:TC, :], res_ps[0:TC, 0, 0:P])
            nc.sync.dma_start(
                out=out.rearrange("(c p) -> c p", p=P), in_=res_t[0:TC, :]
            )

    nc.compile()
    return nc


def _reduce_groups(nc, tc, pools, qk_ps, qk_src_fn, out):
    """General-path phase-C+finale: tanh/exp/mul-reduce over qkT groups,
    then out = num/(den+EPS), PE-transposed for a contiguous output DMA."""
    epool, scrpool, apool, ppool, ident = pools
    den_all = apool.tile([P, TC], F32, tag="den_all")
    num_all = apool.tile([P, TC], F32, tag="num_all")
    for t_i in range(TC):
        den2 = ppool.tile([P, NQH], F32, tag="den2")
        num2 = ppool.tile([P, NQH], F32, tag="num2")
        for h in range(NQH):
            qk = qk_ps.tile([P, QH], F32, tag="qk")
            qk_src_fn(qk, t_i, h)
            th = epool.tile([P, QH], F32, tag="th")
            nc.scalar.activation(out=th, in_=qk, func=AF.Tanh)
            w = epool.tile([P, QH], F32, tag="w")
            nc.scalar.activation(
                out=w, in_=th, func=AF.Exp, accum_out=den2[:, h:h + 1]
            )
            scr = scrpool.tile([P, QH], F32, tag="scr")
            nc.vector.scalar_tensor_tensor(
                out=scr, in0=w, scalar=1.0, in1=qk,
                op0=OP.mult, op1=OP.mult, accum_out=num2[:, h:h + 1],
            )
        nc.vector.tensor_add(den_all[:, t_i:t_i + 1], den2[:, 0:1], den2[:, 1:2])
        nc.vector.tensor_add(num_all[:, t_i:t_i + 1], num2[:, 0:1], num2[:, 1:2])

    den_eps = apool.tile([P, TC], F32, tag="den_eps")
    nc.vector.tensor_scalar_add(den_eps, den_all, EPS)
    recip = apool.tile([P, TC], F32, tag="recip")
    nc.vector.reciprocal(recip, den_eps)
    res = apool.tile([P, TC], F32, tag="res")
    nc.vector.tensor_mul(res, num_all, recip)
    res_ps = qk_ps.tile([P, P], F32, tag="qk")
    nc.tensor.transpose(res_ps[0:TC, :], res, ident)
    res_t = apool.tile([P, P], F32, tag="res_t")
    nc.vector.tensor_copy(res_t[0:TC, :], res_ps[0:TC, :])
    nc.sync.dma_start(out=out.rearrange("(c p) -> c p", p=P), in_=res_t[0:TC, :])


def _build_general():
    """Nonzero-bias build: explicit q/k projections with bias, then qk."""
    nc = bacc.Bacc("TRN2", target_bir_lowering=False, debug=False)

    x1t = nc.dram_tensor("x1t", [D, S], F32R, kind="ExternalInput").ap()
    x2t = nc.dram_tensor("x2t", [D, S], F32R, kind="ExternalInput").ap()
    wq = nc.dram_tensor("wq", [D, D], F32R, kind="ExternalInput").ap()
    wk = nc.dram_tensor("wk", [D, D], F32R, kind="ExternalInput").ap()
    bq = nc.dram_tensor("bq", [D], F32, kind="ExternalInput").ap()
    bk = nc.dram_tensor("bk", [D], F32, kind="ExternalInput").ap()
    out = nc.dram_tensor("out", [S], F32, kind="ExternalOutput").ap()

    with tile.TileContext(nc) as tc:
        with (
            tc.tile_pool(name="weights", bufs=1) as wpool,
            tc.tile_pool(name="big", bufs=1) as bigpool,
            tc.tile_pool(name="xin", bufs=2) as xpool,
            tc.tile_pool(name="elem", bufs=2) as epool,
            tc.tile_pool(name="scrp", bufs=1) as scrpool,
            tc.tile_pool(name="accs", bufs=1) as apool,
            tc.tile_pool(name="parts", bufs=2) as ppool,
            tc.tile_pool(name="pp", bufs=2, space="PSUM") as proj_ps,
            tc.tile_pool(name="qkp", bufs=3, space="PSUM") as qk_ps,
        ):
            wq_sb = wpool.tile([P, DC, D], F32R, tag="wq")
            wk_sb = wpool.tile([P, DC, D], F32R, tag="wk")
            nc.sync.dma_start(out=wq_sb, in_=wq.rearrange("(c p) d -> p c d", p=P))
            nc.sync.dma_start(out=wk_sb, in_=wk.rearrange("(c p) d -> p c d", p=P))
            bq_sb = wpool.tile([P, DC], F32, tag="bq")
            bk_sb = wpool.tile([P, DC], F32, tag="bk")
            nc.sync.dma_start(out=bq_sb, in_=bq.rearrange("(c p) -> p c", p=P))
            nc.sync.dma_start(out=bk_sb, in_=bk.rearrange("(c p) -> p c", p=P))
            ident = wpool.tile([P, P], F32, tag="ident")
            make_identity(nc, ident)

            qt_sb = bigpool.tile([P, DC, S], F32R, tag="qt")
            kt_sb = bigpool.tile([P, DC, S], F32R, tag="kt")

            for xin, w_sb, b_sb, dst, dma_eng in (
                (x1t, wq_sb, bq_sb, qt_sb, nc.scalar),
                (x2t, wk_sb, bk_sb, kt_sb, nc.sync),
            ):
                for sb_i in range(NSB):
                    xblk = xpool.tile([P, DC, SBLK], F32R, tag="xblk")
                    dma_eng.dma_start(
                        out=xblk,
                        in_=xin[:, sb_i * SBLK:(sb_i + 1) * SBLK].rearrange(
                            "(c p) s -> p c s", p=P
                        ),
                    )
                    for e_j in range(DC):
                        pp = proj_ps.tile([P, SBLK], F32, tag="pp")
                        for d_i in range(DC):
                            nc.tensor.matmul(
                                pp,
                                w_sb[:, d_i, e_j * P:(e_j + 1) * P],
                                xblk[:, d_i, :],
                                start=(d_i == 0),
                                stop=(d_i == DC - 1),
                            )
                        nc.scalar.activation(
                            out=dst[:, e_j, sb_i * SBLK:(sb_i + 1) * SBLK],
                            in_=pp, func=AF.Identity,
                            bias=b_sb[:, e_j:e_j + 1], scale=1.0,
                        )

            def qk_group(qk, t_i, h_i):
                for n in range(QH // SBLK):
                    s0 = h_i * QH + n * SBLK
                    for e_i in range(DC):
                        nc.tensor.matmul(
                            qk[:, n * SBLK:(n + 1) * SBLK],
                            kt_sb[:, e_i, t_i * P:(t_i + 1) * P],
                            qt_sb[:, e_i, s0:s0 + SBLK],
                            start=(e_i == 0),
                            stop=(e_i == DC - 1),
                        )

            _reduce_groups(
                nc, tc, (epool, scrpool, apool, ppool, ident), qk_ps, qk_group, out
            )

    nc.compile()
    return nc


def kernel(x1, x2, Wq, bq, Wk, bk, trace=False):
    x1 = np.ascontiguousarray(np.asarray(x1, dtype=np.float32))
    x2 = np.ascontiguousarray(np.asarray(x2, dtype=np.float32))
    Wq = np.ascontiguousarray(np.asarray(Wq, dtype=np.float32))
    Wk = np.ascontiguousarray(np.asarray(Wk, dtype=np.float32))
    bq = np.ascontiguousarray(np.asarray(bq, dtype=np.float32))
    bk = np.ascontiguousarray(np.asarray(bk, dtype=np.float32))

    x1t = np.ascontiguousarray(x1.transpose(0, 2, 1))  # [B, D, S]
    x2t = np.ascontiguousarray(x2.transpose(0, 2, 1))
    cores = list(range(B))

    fast = not (bq.any() or bk.any())
    if fast:
        if "nc_fast" not in _CACHE:
            _CACHE["nc_fast"] = _build_fast()
        nc = _CACHE["nc_fast"]
        f8 = ml_dtypes.float8_e4m3
        h8 = np.ascontiguousarray((Wk @ Wq.T).astype(f8))
        x1t8 = np.ascontiguousarray(x1t.astype(f8))
        x2t8 = np.ascontiguousarray(x2t.astype(f8))
        in_maps = [{"x1t": x1t8[c], "x2t": x2t8[c], "h": h8} for c in cores]
    else:
        if "nc_general" not in _CACHE:
            _CACHE["nc_general"] = _build_general()
        nc = _CACHE["nc_general"]
        in_maps = [
            {"x1t": x1t[c], "x2t": x2t[c], "wq": Wq, "wk": Wk, "bq": bq, "bk": bk}
            for c in cores
        ]
    res = run_bass_kernel_spmd(nc, in_maps, cores, trace=trace)
    _CACHE["last_results"] = res
    return np.stack([res.results[c]["out"] for c in cores])
